# revision 30
# baseline (speedup 1.0000x reference)
"""Liteformer fast attention kernel for Trainium2 (8 NeuronCores), v2.

Math (per (b,h) head, N=8192 tokens, C=K=E=64, m=256 anchors):
    xhat = qk / ||qk||_row
    phi  = tanh((xhat @ anchor.T) @ W_hash) = tanh(xhat @ G),  G = anchor.T @ W_hash  [64,64]
    kcum = phi.sum(axis=0)                                  [64]
    ctx  = phi.T @ v                                        [64,64]
    out  = (phi @ ctx + 65*v) / (phi @ kcum + 8192*65)[:,None]

Sharding: B*H = 32 heads split 4-per-core across 8 cores (fully independent).

v2 engine plan (per 1024-token block; token(blk,p,a) = (blk*128+p)*8+a):
  SP  : x loads (8x256KB/head, prefetched), out stores (256KB/blk)
  POOL: v cast-loads fp32->fp16 (SWDGE), rsqrt via Quake+2 Newton, final mul
  ACT : Square (norms), Tanh phiT (+accum_out -> kcum), Tanh phi, denom bias
        -- single table set (exp_and_others), no table swaps
  DVE : reduce (norms), xn = x*rs (alternating with POOL), xt psum->sbuf copy,
        reciprocal, pass2 stt (65v+numer)
  PE  : 4 transposes, s1T row-tiled pair, s1N x4, ctx x8 col-tiled 2-way,
        pass2 x4 into one 2-bank psum tile (stride-65 numer/denom extraction)
"""

import os
import sys

sys.path.insert(0, "/opt/trn_rl_repo")

DEBUG_TAPS = os.environ.get("KERNEL_DEBUG_TAPS", "0") == "1"

from contextlib import ExitStack

import numpy as np

import concourse.bass as bass
import concourse.tile as tile
from concourse import bacc, mybir
from concourse.bass_utils import run_bass_kernel_spmd
from concourse.masks import make_identity

B, H, N, C = 2, 16, 8192, 64
NBITS = 64
BIAS = NBITS + 1  # 65
DENOM_BIAS = float(N) * BIAS  # 532480
HEADS_PER_CORE = (B * H) // 8  # 4
NBLK = N // 1024  # 8 blocks of 1024 tokens per head
FP32 = mybir.dt.float32
FP16 = mybir.dt.float16
U32 = mybir.dt.uint32
AX = mybir.AxisListType
AF = mybir.ActivationFunctionType
ALU = mybir.AluOpType

RSQRT_MAGIC = 0x5F3759DF


def bcast(ap, n):
    """Append a zero-stride broadcast dim of size n to an AP."""
    return bass.AP(tensor=ap.tensor, offset=ap.offset, ap=ap.ap + [[0, n]])


def alloc_head_tiles(persist):
    specs = {
        "x_h": ([128, NBLK, 8, 64], FP32),
        "v16": ([128, NBLK, 8, 64], FP16),
        "phiT": ([128, NBLK, 512], FP16),
        "nsq": ([128, NBLK, 8], FP32),
        "rs": ([128, NBLK, 8], FP32),
        "qt0": ([128, NBLK, 8], FP32),
        "qt1": ([128, NBLK, 8], FP32),
        "kcp": ([128, NBLK], FP32),
        "g2": ([128, 128], FP16),
        "cc2": ([128, 130], FP16),
        "cksb": ([128, 65], FP16),
        "a_sb": ([128, 2, 64], FP32),
        "w_sb": ([128, 2, 64], FP32),
    }
    return {
        k: persist.tile(shape, dt, tag=k, name=k) for k, (shape, dt) in specs.items()
    }


def emit_loads(nc, ht, qk_h, v_h, a_h, w_h):
    qk_blk = qk_h.rearrange("(blk p a) c -> blk p a c", p=128, a=8)
    v_blk = v_h.rearrange("(blk p a) c -> blk p a c", p=128, a=8)
    for blk in range(NBLK):
        nc.scalar.dma_start(ht["x_h"][:, blk], qk_blk[blk])  # HWDGE queue #2
        nc.gpsimd.dma_start(ht["v16"][:, blk], v_blk[blk])  # SWDGE cast fp32->fp16
    nc.sync.dma_start(ht["a_sb"][:], a_h.rearrange("(t p) c -> p t c", p=128))
    nc.sync.dma_start(ht["w_sb"][:], w_h.rearrange("(t p) c -> p t c", p=128))


def build_head(tc, pools, consts, ht, out_h, blk_eng, prefetch_next, taps=None):
    nc = tc.nc
    temps, persist, ps_xt, ps_p1, ps_ctx, ps_o, ps_small = pools
    ident, stack2, dbias = consts[0], consts[1], consts[2]

    x_h = ht["x_h"]
    v16 = ht["v16"]
    phiT = ht["phiT"]
    nsq = ht["nsq"]
    rs = ht["rs"]
    qt0 = ht["qt0"]
    qt1 = ht["qt1"]
    kcp = ht["kcp"]
    g2 = ht["g2"]
    cc2 = ht["cc2"]
    cksb = ht["cksb"]
    a_sb = ht["a_sb"]
    w_sb = ht["w_sb"]

    out_blk = out_h.rearrange("(blk p a) c -> blk p (a c)", p=128, a=8)

    # ---------------- pass 1a: squared row norms -------------------------
    for blk in range(NBLK):
        sq16 = temps.tile([128, 8, 64], FP16, tag="sq16")
        nc.scalar.activation(sq16[:], x_h[:, blk], AF.Square)
        nc.vector.reduce_sum(nsq[:, blk], sq16[:], axis=AX.X)

    # ---------------- rsqrt via Quake + 2 Newton, two halves -------------
    ge = nc.vector
    magic = consts[3]
    for half in range(2):
        s = slice(half * 4, half * 4 + 4)
        nf = nsq[:, s].rearrange("p b a -> p (b a)")  # [128, 32] fp32
        r0f = qt0[:, s].rearrange("p b a -> p (b a)")
        t1f = qt1[:, s].rearrange("p b a -> p (b a)")
        rsf = rs[:, s].rearrange("p b a -> p (b a)")
        # r0 = bitcast(MAGIC - (bitcast(nsq) >> 1))
        ge.tensor_scalar(
            t1f.bitcast(U32), nf.bitcast(U32), 1, None, ALU.arith_shift_right
        )
        ge.tensor_tensor(
            r0f.bitcast(U32).rearrange("p (o a) -> p o a", o=1),
            bcast(magic[:], 32),
            t1f.bitcast(U32).rearrange("p (o a) -> p o a", o=1),
            ALU.subtract,
        )
        # Newton iter 1: r1 = r0 * (1.5 - 0.5 * nsq * r0^2)   -> rsf
        ge.tensor_tensor(t1f, r0f, r0f, ALU.mult)
        ge.tensor_tensor(t1f, t1f, nf, ALU.mult)
        ge.tensor_scalar(t1f, t1f, -0.5, 1.5, ALU.mult, op1=ALU.add)
        ge.tensor_tensor(rsf, r0f, t1f, ALU.mult)
        # Newton iter 2: rs = r1 * (1.5 - 0.5 * nsq * r1^2)
        ge.tensor_tensor(t1f, rsf, rsf, ALU.mult)
        ge.tensor_tensor(t1f, t1f, nf, ALU.mult)
        ge.tensor_scalar(t1f, t1f, -0.5, 1.5, ALU.mult, op1=ALU.add)
        ge.tensor_tensor(rsf, rsf, t1f, ALU.mult)

    # ---------------- G = anchor.T @ W_hash ; g2 block-diag fp16 ---------
    # (after pass 1a so the norm chain owns the early ACT/DVE slots)
    gk_ps = ps_small.tile([64, 65], FP32, tag="gk_ps")
    for t in range(2):
        nc.tensor.matmul(
            gk_ps[:, 0:64], a_sb[:, t, :], w_sb[:, t, :], start=(t == 0), stop=(t == 1)
        )
    nc.vector.memset(g2[:], 0.0)
    nc.vector.tensor_copy(g2[0:64, 0:64], gk_ps[:, 0:64])
    nc.vector.tensor_copy(g2[64:128, 64:128], gk_ps[:, 0:64])

    if taps is not None:
        nc.sync.dma_start(taps["rs"], rsf)
        nc.sync.dma_start(taps["nsq"], nf)

    # ---------------- pass 1b: phi production + ctx ----------------------
    # ctxAB: partitions 0:64 accumulate even-a ctx, 64:128 odd-a (col-tiled)
    ctxAB = ps_ctx.tile([128, 64], FP32, tag="ctxAB")
    for blk in range(NBLK):
        # xn = x * rs (per-token row scale), fp16; alternate DVE/POOL
        xn = temps.tile([128, 8, 64], FP16, tag="xn")
        eng = blk_eng(blk)
        eng.tensor_tensor(
            xn[:],
            x_h[:, blk],
            bcast(rs[:, blk].rearrange("p (a o) -> p a o", o=1), 64),
            ALU.mult,
        )

        # 4x PE transpose -> xt double-decker fp16
        xt_ps = ps_xt.tile([128, 512], FP16, tag="xt_ps")
        xn2 = xn[:].rearrange("p a c -> p (a c)")
        for j in range(4):
            nc.tensor.transpose(
                xt_ps[:, j * 128 : (j + 1) * 128],
                xn2[:, j * 128 : (j + 1) * 128],
                ident[:],
            )
        xt = temps.tile([128, 512], FP16, tag="xt")
        nc.vector.tensor_copy(xt[:], xt_ps[:])

        # s1T: phiT = tanh(G^T xhat^T), two concurrent row-tiled matmuls
        pT_ps = ps_p1.tile([128, 512], FP32, tag="p1_ps")
        nc.tensor.matmul(pT_ps[0:64, :], g2[0:64, 0:64], xt[0:64, :], start=True, stop=True)
        nc.tensor.matmul(pT_ps[64:128, :], g2[64:128, 64:128], xt[64:128, :], start=True, stop=True)
        # tanh with free-dim accumulation -> per-(deck,k) kcum contribution
        nc.scalar.activation(
            phiT[:, blk, :], pT_ps[:], AF.Tanh, accum_out=kcp[:, blk : blk + 1]
        )

        # s1N: phi natural [p, (a,k)] via 4 matmuls against block-diag g2
        pN_ps = ps_p1.tile([128, 512], FP32, tag="p1_ps")
        for j in range(4):
            nc.tensor.matmul(
                pN_ps[:, j * 128 : (j + 1) * 128],
                xt[:, j * 128 : (j + 1) * 128],
                g2[:],
                start=True,
                stop=True,
            )
        phi = temps.tile([128, 8, 64], FP16, tag="phi")
        nc.scalar.activation(phi[:].rearrange("p a c -> p (a c)"), pN_ps[:], AF.Tanh)
        if taps is not None and blk == 0:
            nc.sync.dma_start(taps["phi0"], phi[:].rearrange("p a c -> p (a c)"))
            nc.sync.dma_start(taps["phiT0"], phiT[:, 0, :])

        # ctx += phi_a.T @ v_a ; even a -> rows 0:64, odd a -> rows 64:128
        for a in range(8):
            half = a % 2
            nc.tensor.matmul(
                ctxAB[half * 64 : half * 64 + 64, :],
                phi[:, a, :],
                v16[:, blk, a, :],
                start=(blk == 0 and a == half),
                stop=(blk == NBLK - 1 and a == 6 + half),
                # sim's zero-region group check ignores partition base; the two
                # col-tiled halves are disjoint partition ranges on HW
                skip_group_check=True,
            )

    # prefetch the next head's inputs while this head's pass 2 runs
    if prefetch_next is not None:
        prefetch_next()

    # ---------------- head finalize: [ctx|kcum] + cc2 --------------------
    kc1 = temps.tile([128, 1], FP32, tag="kc1")
    nc.vector.reduce_sum(kc1[:], kcp[:], axis=AX.X)
    nc.scalar.copy(cksb[:, 0:64], ctxAB[:])
    nc.scalar.copy(cksb[:, 64:65], kc1[:])
    ck_ps = ps_small.tile([64, 65], FP32, tag="gk_ps")
    nc.tensor.matmul(ck_ps[:], stack2[:], cksb[:], start=True, stop=True)
    nc.vector.memset(cc2[:], 0.0)
    nc.scalar.copy(cc2[0:64, 0:65], ck_ps[:])
    nc.scalar.copy(cc2[64:128, 65:130], ck_ps[:])
    if taps is not None:
        nc.sync.dma_start(taps["cc2"], cc2[:])
        nc.sync.dma_start(taps["g2"], g2[:])

    # ---------------- pass 2 ---------------------------------------------
    # o_ab is one 2-bank psum tile; matmul outputs at col {0,130,512,642}
    # so numer/denom extraction is affine: off = 512*b + 65*g (g in 0..3).
    OFFS = (0, 130, 512, 642)
    for blk in range(NBLK):
        o_ab = ps_o.tile([128, 1024], FP32, tag="o_ab")
        for j in range(4):
            nc.tensor.matmul(
                o_ab[:, OFFS[j] : OFFS[j] + 130],
                phiT[:, blk, j * 128 : (j + 1) * 128],
                cc2[:],
                start=True,
                stop=True,
            )
        t = o_ab[:]
        denom = bass.AP(
            tensor=t.tensor, offset=t.offset + 64, ap=[t.ap[0], [512, 2], [65, 4]]
        )
        dnb = temps.tile([128, 8], FP32, tag="dnb")
        nc.scalar.activation(
            dnb[:].rearrange("p (b g) -> p b g", b=2), denom, AF.Identity, bias=dbias[:]
        )
        rec = temps.tile([128, 8], FP32, tag="rec")
        nc.vector.reciprocal(rec[:], dnb[:])
        # t_sb = 65*v + numer  (per psum bank: STT inputs are limited to 3D)
        t_sb = temps.tile([128, 8, 64], FP32, tag="t_sb")
        for b in range(2):
            numer_b = bass.AP(
                tensor=t.tensor, offset=t.offset + 512 * b, ap=[t.ap[0], [65, 4], [1, 64]]
            )
            nc.vector.scalar_tensor_tensor(
                out=t_sb[:, 4 * b : 4 * b + 4, :],
                in0=v16[:, blk, 4 * b : 4 * b + 4, :],
                scalar=float(BIAS),
                in1=numer_b,
                op0=ALU.mult,
                op1=ALU.add,
            )
        o_sb = temps.tile([128, 8, 64], FP32, tag="o_sb")
        eng = blk_eng(blk + 1)  # opposite parity from xn
        eng.tensor_tensor(
            o_sb[:],
            t_sb[:],
            bcast(rec[:].rearrange("p (a o) -> p a o", o=1), 64),
            ALU.mult,
        )
        nc.sync.dma_start(out_blk[blk], o_sb[:].rearrange("p a c -> p (a c)"))


def build_core(tc, pools, consts, qk_ap, v_ap, a_ap, w_ap, out_ap, heads):
    nc = tc.nc

    def blk_eng(blk):
        return nc.vector if blk % 2 == 0 else nc.gpsimd

    taps = None
    if DEBUG_TAPS:
        taps = {
            "rs": nc.dram_tensor("dbg_rs", (128, 64), FP32, kind="ExternalOutput").ap(),
            "nsq": nc.dram_tensor("dbg_nsq", (128, 64), FP32, kind="ExternalOutput").ap(),
            "phi0": nc.dram_tensor("dbg_phi0", (128, 512), FP16, kind="ExternalOutput").ap(),
            "phiT0": nc.dram_tensor("dbg_phiT0", (128, 512), FP16, kind="ExternalOutput").ap(),
            "cc2": nc.dram_tensor("dbg_cc2", (128, 130), FP16, kind="ExternalOutput").ap(),
            "g2": nc.dram_tensor("dbg_g2", (128, 128), FP16, kind="ExternalOutput").ap(),
        }

    persist = pools[1]
    tiles = [alloc_head_tiles(persist) for _ in range(heads)]
    emit_loads(nc, tiles[0], qk_ap[0], v_ap[0], a_ap[0], w_ap[0])
    for h in range(heads):
        if h + 1 < heads:
            hn = h + 1

            def prefetch_next(hn=hn):
                emit_loads(nc, tiles[hn], qk_ap[hn], v_ap[hn], a_ap[hn], w_ap[hn])
        else:
            prefetch_next = None
        build_head(
            tc,
            pools,
            consts,
            tiles[h],
            out_ap[h],
            blk_eng,
            prefetch_next,
            taps=taps if h == 0 else None,
        )


def build_bass(heads=HEADS_PER_CORE, repeat=1):
    nc = bacc.Bacc("TRN2", target_bir_lowering=False, debug=False, num_devices=8)
    hp = heads
    qk_ap = nc.dram_tensor("qk", (hp, N, C), FP32, kind="ExternalInput").ap()
    v_ap = nc.dram_tensor("v", (hp, N, C), FP32, kind="ExternalInput").ap()
    a_ap = nc.dram_tensor("anchor", (hp, 256, C), FP32, kind="ExternalInput").ap()
    w_ap = nc.dram_tensor("W_hash", (hp, 256, NBITS), FP32, kind="ExternalInput").ap()
    out_ap = nc.dram_tensor("out", (hp, N, C), FP32, kind="ExternalOutput").ap()

    with tile.TileContext(nc) as tc:
        with ExitStack() as ctx:
            singles = ctx.enter_context(tc.tile_pool(name="singles", bufs=1))
            temps = ctx.enter_context(tc.tile_pool(name="temps", bufs=3))
            persist = ctx.enter_context(tc.tile_pool(name="persist", bufs=2))
            ps_xt = ctx.enter_context(tc.tile_pool(name="ps_xt", bufs=1, space="PSUM"))
            ps_p1 = ctx.enter_context(tc.tile_pool(name="ps_p1", bufs=1, space="PSUM"))
            ps_ctx = ctx.enter_context(tc.tile_pool(name="ps_ctx", bufs=1, space="PSUM"))
            ps_o = ctx.enter_context(tc.tile_pool(name="ps_o", bufs=2, space="PSUM"))
            ps_small = ctx.enter_context(
                tc.tile_pool(name="ps_small", bufs=1, space="PSUM")
            )
            pools = (temps, persist, ps_xt, ps_p1, ps_ctx, ps_o, ps_small)

            ident = singles.tile([128, 128], FP16)
            make_identity(nc, ident[:])
            stack2 = singles.tile([128, 64], FP16)
            nc.scalar.copy(stack2[0:64, :], ident[0:64, 0:64])
            nc.scalar.copy(stack2[64:128, :], ident[0:64, 0:64])
            dbias = singles.tile([128, 1], FP32)
            nc.vector.memset(dbias[:], DENOM_BIAS)
            magic = singles.tile([128, 1], U32)
            nc.vector._memset_packed(magic[:], RSQRT_MAGIC)
            consts = (ident, stack2, dbias, magic)

            if repeat == 1:
                build_core(tc, pools, consts, qk_ap, v_ap, a_ap, w_ap, out_ap, heads)
            else:
                with tc.For_i(0, repeat, 1):
                    build_core(
                        tc, pools, consts, qk_ap, v_ap, a_ap, w_ap, out_ap, heads
                    )
    nc.compile()
    return nc


_NC_CACHE = None
_RUN_KWARGS = {}
_LAST_RESULTS = None


def kernel(qk, v, anchor, W_hash):
    global _NC_CACHE
    if _NC_CACHE is None:
        _NC_CACHE = build_bass()
    nc = _NC_CACHE

    qk = np.ascontiguousarray(qk, dtype=np.float32).reshape(B * H, N, C)
    v = np.ascontiguousarray(v, dtype=np.float32).reshape(B * H, N, C)
    anchor = np.ascontiguousarray(anchor, dtype=np.float32)
    W_hash = np.ascontiguousarray(W_hash, dtype=np.float32)

    in_maps = []
    for core in range(8):
        bh = np.arange(core * HEADS_PER_CORE, (core + 1) * HEADS_PER_CORE)
        h_idx = bh % H
        in_maps.append(
            {
                "qk": qk[bh],
                "v": v[bh],
                "anchor": np.ascontiguousarray(anchor[h_idx]),
                "W_hash": np.ascontiguousarray(W_hash[h_idx]),
            }
        )

    res = run_bass_kernel_spmd(nc, in_maps, core_ids=list(range(8)), **_RUN_KWARGS)
    global _LAST_RESULTS
    _LAST_RESULTS = res
    out = np.concatenate([res.results[c]["out"] for c in range(8)], axis=0)
    return out.reshape(B, H, N, C)


# revision 31
# speedup vs baseline: 1.1604x; 1.1604x over previous
"""Liteformer fast attention kernel for Trainium2 (8 NeuronCores), v2.

Math (per (b,h) head, N=8192 tokens, C=K=E=64, m=256 anchors):
    xhat = qk / ||qk||_row
    phi  = tanh((xhat @ anchor.T) @ W_hash) = tanh(xhat @ G),  G = anchor.T @ W_hash  [64,64]
    kcum = phi.sum(axis=0)                                  [64]
    ctx  = phi.T @ v                                        [64,64]
    out  = (phi @ ctx + 65*v) / (phi @ kcum + 8192*65)[:,None]

Sharding: B*H = 32 heads split 4-per-core across 8 cores (fully independent).

v2 engine plan (per 1024-token block; token(blk,p,a) = (blk*128+p)*8+a):
  SP  : x loads (8x256KB/head, prefetched), out stores (256KB/blk)
  POOL: v cast-loads fp32->fp16 (SWDGE), rsqrt via Quake+2 Newton, final mul
  ACT : Square (norms), Tanh phiT (+accum_out -> kcum), Tanh phi, denom bias
        -- single table set (exp_and_others), no table swaps
  DVE : reduce (norms), xn = x*rs (alternating with POOL), xt psum->sbuf copy,
        reciprocal, pass2 stt (65v+numer)
  PE  : 4 transposes, s1T row-tiled pair, s1N x4, ctx x8 col-tiled 2-way,
        pass2 x4 into one 2-bank psum tile (stride-65 numer/denom extraction)
"""

import os
import sys

sys.path.insert(0, "/opt/trn_rl_repo")

DEBUG_TAPS = os.environ.get("KERNEL_DEBUG_TAPS", "0") == "1"

from contextlib import ExitStack

import numpy as np

import concourse.bass as bass
import concourse.tile as tile
from concourse import bacc, mybir
from concourse.bass_utils import run_bass_kernel_spmd
from concourse.masks import make_identity

B, H, N, C = 2, 16, 8192, 64
NBITS = 64
BIAS = NBITS + 1  # 65
DENOM_BIAS = float(N) * BIAS  # 532480
HEADS_PER_CORE = (B * H) // 8  # 4
NBLK = N // 1024  # 8 blocks of 1024 tokens per head
FP32 = mybir.dt.float32
FP16 = mybir.dt.float16
U32 = mybir.dt.uint32
AX = mybir.AxisListType
AF = mybir.ActivationFunctionType
ALU = mybir.AluOpType

RSQRT_MAGIC = 0x5F3759DF


def bcast(ap, n):
    """Append a zero-stride broadcast dim of size n to an AP."""
    return bass.AP(tensor=ap.tensor, offset=ap.offset, ap=ap.ap + [[0, n]])


def alloc_head_tiles(persist):
    specs = {
        "x_h": ([128, NBLK, 8, 64], FP32),
        "v16": ([128, NBLK, 8, 64], FP16),
        "phiT": ([128, NBLK, 512], FP16),
        "nsq": ([128, NBLK, 8], FP32),
        "rs": ([128, NBLK, 8], FP32),
        "qt0": ([128, NBLK, 8], FP32),
        "qt1": ([128, NBLK, 8], FP32),
        "kcp": ([128, NBLK], FP32),
        "g2": ([128, 128], FP16),
        "cc2": ([128, 130], FP16),
        "cksb": ([128, 65], FP16),
        "a_sb": ([128, 2, 64], FP32),
        "w_sb": ([128, 2, 64], FP32),
    }
    return {
        k: persist.tile(shape, dt, tag=k, name=k) for k, (shape, dt) in specs.items()
    }


def emit_loads(nc, ht, qk_h, v_h, a_h, w_h):
    qk_blk = qk_h.rearrange("(blk p a) c -> blk p a c", p=128, a=8)
    v_blk = v_h.rearrange("(blk p a) c -> blk p a c", p=128, a=8)
    for blk in range(NBLK):
        nc.sync.dma_start(ht["x_h"][:, blk], qk_blk[blk])
        nc.gpsimd.dma_start(ht["v16"][:, blk], v_blk[blk])  # SWDGE cast fp32->fp16
    nc.sync.dma_start(ht["a_sb"][:], a_h.rearrange("(t p) c -> p t c", p=128))
    nc.sync.dma_start(ht["w_sb"][:], w_h.rearrange("(t p) c -> p t c", p=128))


def build_head(tc, pools, consts, ht, out_h, blk_eng, prefetch_next, taps=None):
    nc = tc.nc
    temps, persist, ps_xt, ps_p1, ps_ctx, ps_o, ps_small = pools
    ident, stack2, dbias = consts[0], consts[1], consts[2]

    x_h = ht["x_h"]
    v16 = ht["v16"]
    phiT = ht["phiT"]
    nsq = ht["nsq"]
    rs = ht["rs"]
    qt0 = ht["qt0"]
    qt1 = ht["qt1"]
    kcp = ht["kcp"]
    g2 = ht["g2"]
    cc2 = ht["cc2"]
    cksb = ht["cksb"]
    a_sb = ht["a_sb"]
    w_sb = ht["w_sb"]

    out_blk = out_h.rearrange("(blk p a) c -> blk p (a c)", p=128, a=8)

    # ---------------- pass 1a: squared row norms -------------------------
    for blk in range(NBLK):
        sq16 = temps.tile([128, 8, 64], FP16, tag="sq16")
        nc.scalar.activation(sq16[:], x_h[:, blk], AF.Square)
        nc.vector.reduce_sum(nsq[:, blk], sq16[:], axis=AX.X)

    # ---------------- rsqrt via Quake + 2 Newton, two halves -------------
    ge = nc.vector
    magic = consts[3]
    for half in range(2):
        s = slice(half * 4, half * 4 + 4)
        nf = nsq[:, s].rearrange("p b a -> p (b a)")  # [128, 32] fp32
        r0f = qt0[:, s].rearrange("p b a -> p (b a)")
        t1f = qt1[:, s].rearrange("p b a -> p (b a)")
        rsf = rs[:, s].rearrange("p b a -> p (b a)")
        # r0 = bitcast(MAGIC - (bitcast(nsq) >> 1))
        ge.tensor_scalar(
            t1f.bitcast(U32), nf.bitcast(U32), 1, None, ALU.arith_shift_right
        )
        ge.tensor_tensor(
            r0f.bitcast(U32).rearrange("p (o a) -> p o a", o=1),
            bcast(magic[:], 32),
            t1f.bitcast(U32).rearrange("p (o a) -> p o a", o=1),
            ALU.subtract,
        )
        # Newton iter 1: r1 = r0 * (1.5 - 0.5 * nsq * r0^2)   -> rsf
        ge.tensor_tensor(t1f, r0f, r0f, ALU.mult)
        ge.tensor_tensor(t1f, t1f, nf, ALU.mult)
        ge.tensor_scalar(t1f, t1f, -0.5, 1.5, ALU.mult, op1=ALU.add)
        ge.tensor_tensor(rsf, r0f, t1f, ALU.mult)
        # Newton iter 2: rs = r1 * (1.5 - 0.5 * nsq * r1^2)
        ge.tensor_tensor(t1f, rsf, rsf, ALU.mult)
        ge.tensor_tensor(t1f, t1f, nf, ALU.mult)
        ge.tensor_scalar(t1f, t1f, -0.5, 1.5, ALU.mult, op1=ALU.add)
        ge.tensor_tensor(rsf, rsf, t1f, ALU.mult)

    # ---------------- G = anchor.T @ W_hash ; g2 block-diag fp16 ---------
    # (after pass 1a so the norm chain owns the early ACT/DVE slots)
    gk_ps = ps_small.tile([64, 65], FP32, tag="gk_ps")
    for t in range(2):
        nc.tensor.matmul(
            gk_ps[:, 0:64], a_sb[:, t, :], w_sb[:, t, :], start=(t == 0), stop=(t == 1)
        )
    nc.vector.memset(g2[:], 0.0)
    nc.vector.tensor_copy(g2[0:64, 0:64], gk_ps[:, 0:64])
    nc.vector.tensor_copy(g2[64:128, 64:128], gk_ps[:, 0:64])

    if taps is not None:
        nc.sync.dma_start(taps["rs"], rsf)
        nc.sync.dma_start(taps["nsq"], nf)

    # ---------------- pass 1b: phi production + ctx ----------------------
    # ctxAB: partitions 0:64 accumulate even-a ctx, 64:128 odd-a (col-tiled)
    ctxAB = ps_ctx.tile([128, 64], FP32, tag="ctxAB")
    for blk in range(NBLK):
        # xn = x * rs (per-token row scale), fp16; alternate DVE/POOL
        xn = temps.tile([128, 8, 64], FP16, tag="xn")
        eng = blk_eng(blk)
        eng.tensor_tensor(
            xn[:],
            x_h[:, blk],
            bcast(rs[:, blk].rearrange("p (a o) -> p a o", o=1), 64),
            ALU.mult,
        )

        # 4x PE transpose -> xt double-decker fp16
        xt_ps = ps_xt.tile([128, 512], FP16, tag="xt_ps")
        xn2 = xn[:].rearrange("p a c -> p (a c)")
        for j in range(4):
            nc.tensor.transpose(
                xt_ps[:, j * 128 : (j + 1) * 128],
                xn2[:, j * 128 : (j + 1) * 128],
                ident[:],
            )
        xt = temps.tile([128, 512], FP16, tag="xt")
        nc.vector.tensor_copy(xt[:], xt_ps[:])

        # s1T: phiT = tanh(G^T xhat^T), two concurrent row-tiled matmuls
        pT_ps = ps_p1.tile([128, 512], FP32, tag="p1_ps")
        nc.tensor.matmul(pT_ps[0:64, :], g2[0:64, 0:64], xt[0:64, :], start=True, stop=True)
        nc.tensor.matmul(pT_ps[64:128, :], g2[64:128, 64:128], xt[64:128, :], start=True, stop=True)
        # tanh with free-dim accumulation -> per-(deck,k) kcum contribution
        nc.scalar.activation(
            phiT[:, blk, :], pT_ps[:], AF.Tanh, accum_out=kcp[:, blk : blk + 1]
        )

        # s1N: phi natural [p, (a,k)] via 4 matmuls against block-diag g2
        pN_ps = ps_p1.tile([128, 512], FP32, tag="p1_ps")
        for j in range(4):
            nc.tensor.matmul(
                pN_ps[:, j * 128 : (j + 1) * 128],
                xt[:, j * 128 : (j + 1) * 128],
                g2[:],
                start=True,
                stop=True,
            )
        phi = temps.tile([128, 8, 64], FP16, tag="phi")
        nc.scalar.activation(phi[:].rearrange("p a c -> p (a c)"), pN_ps[:], AF.Tanh)
        if taps is not None and blk == 0:
            nc.sync.dma_start(taps["phi0"], phi[:].rearrange("p a c -> p (a c)"))
            nc.sync.dma_start(taps["phiT0"], phiT[:, 0, :])

        # ctx += phi_a.T @ v_a ; even a -> rows 0:64, odd a -> rows 64:128
        for a in range(8):
            half = a % 2
            nc.tensor.matmul(
                ctxAB[half * 64 : half * 64 + 64, :],
                phi[:, a, :],
                v16[:, blk, a, :],
                start=(blk == 0 and a == half),
                stop=(blk == NBLK - 1 and a == 6 + half),
                # sim's zero-region group check ignores partition base; the two
                # col-tiled halves are disjoint partition ranges on HW
                skip_group_check=True,
            )

    # prefetch the next head's inputs while this head's pass 2 runs
    if prefetch_next is not None:
        prefetch_next()

    # ---------------- head finalize: [ctx|kcum] + cc2 --------------------
    kc1 = temps.tile([128, 1], FP32, tag="kc1")
    nc.vector.reduce_sum(kc1[:], kcp[:], axis=AX.X)
    nc.scalar.copy(cksb[:, 0:64], ctxAB[:])
    nc.scalar.copy(cksb[:, 64:65], kc1[:])
    ck_ps = ps_small.tile([64, 65], FP32, tag="gk_ps")
    nc.tensor.matmul(ck_ps[:], stack2[:], cksb[:], start=True, stop=True)
    nc.vector.memset(cc2[:], 0.0)
    nc.scalar.copy(cc2[0:64, 0:65], ck_ps[:])
    nc.scalar.copy(cc2[64:128, 65:130], ck_ps[:])
    if taps is not None:
        nc.sync.dma_start(taps["cc2"], cc2[:])
        nc.sync.dma_start(taps["g2"], g2[:])

    # ---------------- pass 2 ---------------------------------------------
    # o_ab is one 2-bank psum tile; matmul outputs at col {0,130,512,642}
    # so numer/denom extraction is affine: off = 512*b + 65*g (g in 0..3).
    OFFS = (0, 130, 512, 642)
    for blk in range(NBLK):
        o_ab = ps_o.tile([128, 1024], FP32, tag="o_ab")
        for j in range(4):
            nc.tensor.matmul(
                o_ab[:, OFFS[j] : OFFS[j] + 130],
                phiT[:, blk, j * 128 : (j + 1) * 128],
                cc2[:],
                start=True,
                stop=True,
            )
        t = o_ab[:]
        denom = bass.AP(
            tensor=t.tensor, offset=t.offset + 64, ap=[t.ap[0], [512, 2], [65, 4]]
        )
        dnb = temps.tile([128, 8], FP32, tag="dnb")
        nc.scalar.activation(
            dnb[:].rearrange("p (b g) -> p b g", b=2), denom, AF.Identity, bias=dbias[:]
        )
        rec = temps.tile([128, 8], FP32, tag="rec")
        nc.vector.reciprocal(rec[:], dnb[:])
        # t_sb = 65*v + numer  (per psum bank: STT inputs are limited to 3D)
        t_sb = temps.tile([128, 8, 64], FP32, tag="t_sb")
        for b in range(2):
            numer_b = bass.AP(
                tensor=t.tensor, offset=t.offset + 512 * b, ap=[t.ap[0], [65, 4], [1, 64]]
            )
            nc.vector.scalar_tensor_tensor(
                out=t_sb[:, 4 * b : 4 * b + 4, :],
                in0=v16[:, blk, 4 * b : 4 * b + 4, :],
                scalar=float(BIAS),
                in1=numer_b,
                op0=ALU.mult,
                op1=ALU.add,
            )
        o_sb = temps.tile([128, 8, 64], FP32, tag="o_sb")
        eng = blk_eng(blk + 1)  # opposite parity from xn
        eng.tensor_tensor(
            o_sb[:],
            t_sb[:],
            bcast(rec[:].rearrange("p (a o) -> p a o", o=1), 64),
            ALU.mult,
        )
        nc.sync.dma_start(out_blk[blk], o_sb[:].rearrange("p a c -> p (a c)"))


def build_core(tc, pools, consts, qk_ap, v_ap, a_ap, w_ap, out_ap, heads):
    nc = tc.nc

    def blk_eng(blk):
        return nc.vector if blk % 2 == 0 else nc.gpsimd

    taps = None
    if DEBUG_TAPS:
        taps = {
            "rs": nc.dram_tensor("dbg_rs", (128, 64), FP32, kind="ExternalOutput").ap(),
            "nsq": nc.dram_tensor("dbg_nsq", (128, 64), FP32, kind="ExternalOutput").ap(),
            "phi0": nc.dram_tensor("dbg_phi0", (128, 512), FP16, kind="ExternalOutput").ap(),
            "phiT0": nc.dram_tensor("dbg_phiT0", (128, 512), FP16, kind="ExternalOutput").ap(),
            "cc2": nc.dram_tensor("dbg_cc2", (128, 130), FP16, kind="ExternalOutput").ap(),
            "g2": nc.dram_tensor("dbg_g2", (128, 128), FP16, kind="ExternalOutput").ap(),
        }

    persist = pools[1]
    tiles = [alloc_head_tiles(persist) for _ in range(heads)]
    emit_loads(nc, tiles[0], qk_ap[0], v_ap[0], a_ap[0], w_ap[0])
    for h in range(heads):
        if h + 1 < heads:
            hn = h + 1

            def prefetch_next(hn=hn):
                emit_loads(nc, tiles[hn], qk_ap[hn], v_ap[hn], a_ap[hn], w_ap[hn])
        else:
            prefetch_next = None
        build_head(
            tc,
            pools,
            consts,
            tiles[h],
            out_ap[h],
            blk_eng,
            prefetch_next,
            taps=taps if h == 0 else None,
        )


def build_bass(heads=HEADS_PER_CORE, repeat=1):
    nc = bacc.Bacc("TRN2", target_bir_lowering=False, debug=False, num_devices=8)
    hp = heads
    qk_ap = nc.dram_tensor("qk", (hp, N, C), FP32, kind="ExternalInput").ap()
    v_ap = nc.dram_tensor("v", (hp, N, C), FP32, kind="ExternalInput").ap()
    a_ap = nc.dram_tensor("anchor", (hp, 256, C), FP32, kind="ExternalInput").ap()
    w_ap = nc.dram_tensor("W_hash", (hp, 256, NBITS), FP32, kind="ExternalInput").ap()
    out_ap = nc.dram_tensor("out", (hp, N, C), FP32, kind="ExternalOutput").ap()

    with tile.TileContext(nc) as tc:
        with ExitStack() as ctx:
            singles = ctx.enter_context(tc.tile_pool(name="singles", bufs=1))
            temps = ctx.enter_context(tc.tile_pool(name="temps", bufs=3))
            persist = ctx.enter_context(tc.tile_pool(name="persist", bufs=2))
            ps_xt = ctx.enter_context(tc.tile_pool(name="ps_xt", bufs=1, space="PSUM"))
            ps_p1 = ctx.enter_context(tc.tile_pool(name="ps_p1", bufs=1, space="PSUM"))
            ps_ctx = ctx.enter_context(tc.tile_pool(name="ps_ctx", bufs=1, space="PSUM"))
            ps_o = ctx.enter_context(tc.tile_pool(name="ps_o", bufs=2, space="PSUM"))
            ps_small = ctx.enter_context(
                tc.tile_pool(name="ps_small", bufs=1, space="PSUM")
            )
            pools = (temps, persist, ps_xt, ps_p1, ps_ctx, ps_o, ps_small)

            ident = singles.tile([128, 128], FP16)
            make_identity(nc, ident[:])
            stack2 = singles.tile([128, 64], FP16)
            nc.scalar.copy(stack2[0:64, :], ident[0:64, 0:64])
            nc.scalar.copy(stack2[64:128, :], ident[0:64, 0:64])
            dbias = singles.tile([128, 1], FP32)
            nc.vector.memset(dbias[:], DENOM_BIAS)
            magic = singles.tile([128, 1], U32)
            nc.vector._memset_packed(magic[:], RSQRT_MAGIC)
            consts = (ident, stack2, dbias, magic)

            if repeat == 1:
                build_core(tc, pools, consts, qk_ap, v_ap, a_ap, w_ap, out_ap, heads)
            else:
                with tc.For_i(0, repeat, 1):
                    build_core(
                        tc, pools, consts, qk_ap, v_ap, a_ap, w_ap, out_ap, heads
                    )
    nc.compile()
    return nc


_NC_CACHE = None
_RUN_KWARGS = {}
_LAST_RESULTS = None


def kernel(qk, v, anchor, W_hash):
    global _NC_CACHE
    if _NC_CACHE is None:
        _NC_CACHE = build_bass()
    nc = _NC_CACHE

    qk = np.ascontiguousarray(qk, dtype=np.float32).reshape(B * H, N, C)
    v = np.ascontiguousarray(v, dtype=np.float32).reshape(B * H, N, C)
    anchor = np.ascontiguousarray(anchor, dtype=np.float32)
    W_hash = np.ascontiguousarray(W_hash, dtype=np.float32)

    in_maps = []
    for core in range(8):
        bh = np.arange(core * HEADS_PER_CORE, (core + 1) * HEADS_PER_CORE)
        h_idx = bh % H
        in_maps.append(
            {
                "qk": qk[bh],
                "v": v[bh],
                "anchor": np.ascontiguousarray(anchor[h_idx]),
                "W_hash": np.ascontiguousarray(W_hash[h_idx]),
            }
        )

    res = run_bass_kernel_spmd(nc, in_maps, core_ids=list(range(8)), **_RUN_KWARGS)
    global _LAST_RESULTS
    _LAST_RESULTS = res
    out = np.concatenate([res.results[c]["out"] for c in range(8)], axis=0)
    return out.reshape(B, H, N, C)


# revision 35
# speedup vs baseline: 1.4077x; 1.2132x over previous
"""Liteformer fast attention kernel for Trainium2 (8 NeuronCores), v2.

Math (per (b,h) head, N=8192 tokens, C=K=E=64, m=256 anchors):
    xhat = qk / ||qk||_row
    phi  = tanh((xhat @ anchor.T) @ W_hash) = tanh(xhat @ G),  G = anchor.T @ W_hash  [64,64]
    kcum = phi.sum(axis=0)                                  [64]
    ctx  = phi.T @ v                                        [64,64]
    out  = (phi @ ctx + 65*v) / (phi @ kcum + 8192*65)[:,None]

Sharding: B*H = 32 heads split 4-per-core across 8 cores (fully independent).

v2 engine plan (per 1024-token block; token(blk,p,a) = (blk*128+p)*8+a):
  SP  : x loads (8x256KB/head, prefetched), out stores (256KB/blk)
  POOL: v cast-loads fp32->fp16 (SWDGE), rsqrt via Quake+2 Newton, final mul
  ACT : Square (norms), Tanh phiT (+accum_out -> kcum), Tanh phi, denom bias
        -- single table set (exp_and_others), no table swaps
  DVE : reduce (norms), xn = x*rs (alternating with POOL), xt psum->sbuf copy,
        reciprocal, pass2 stt (65v+numer)
  PE  : 4 transposes, s1T row-tiled pair, s1N x4, ctx x8 col-tiled 2-way,
        pass2 x4 into one 2-bank psum tile (stride-65 numer/denom extraction)
"""

import os
import sys

sys.path.insert(0, "/opt/trn_rl_repo")

DEBUG_TAPS = os.environ.get("KERNEL_DEBUG_TAPS", "0") == "1"

from contextlib import ExitStack

import numpy as np

import concourse.bass as bass
import concourse.tile as tile
from concourse import bacc, mybir
from concourse.bass_utils import run_bass_kernel_spmd
from concourse.masks import make_identity

B, H, N, C = 2, 16, 8192, 64
NBITS = 64
BIAS = NBITS + 1  # 65
DENOM_BIAS = float(N) * BIAS  # 532480
HEADS_PER_CORE = (B * H) // 8  # 4
NBLK = N // 1024  # 8 blocks of 1024 tokens per head
FP32 = mybir.dt.float32
FP16 = mybir.dt.float16
U32 = mybir.dt.uint32
AX = mybir.AxisListType
AF = mybir.ActivationFunctionType
ALU = mybir.AluOpType

RSQRT_MAGIC = 0x5F3759DF


def bcast(ap, n):
    """Append a zero-stride broadcast dim of size n to an AP."""
    return bass.AP(tensor=ap.tensor, offset=ap.offset, ap=ap.ap + [[0, n]])


def alloc_head_tiles(persist):
    specs = {
        "x_h": ([128, NBLK, 8, 64], FP32),
        "v16": ([128, NBLK, 8, 64], FP16),
        "phiT": ([128, NBLK, 512], FP16),
        "nsq": ([128, NBLK, 8], FP32),
        "rs": ([128, NBLK, 8], FP32),
        "qt0": ([128, NBLK, 8], FP32),
        "qt1": ([128, NBLK, 8], FP32),
        "kcp": ([128, NBLK], FP32),
        "g2": ([128, 128], FP16),
        "cc2": ([128, 130], FP16),
        "cksb": ([128, 65], FP16),
        "a_sb": ([128, 2, 64], FP32),
        "w_sb": ([128, 2, 64], FP32),
    }
    return {
        k: persist.tile(shape, dt, tag=k, name=k) for k, (shape, dt) in specs.items()
    }


def emit_loads(nc, ht, qk_h, v_h, a_h, w_h):
    qk_blk = qk_h.rearrange("(blk p a) c -> blk p a c", p=128, a=8)
    v_blk = v_h.rearrange("(blk p a) c -> blk p a c", p=128, a=8)
    for blk in range(NBLK):
        nc.sync.dma_start(ht["x_h"][:, blk], qk_blk[blk])
        nc.gpsimd.dma_start(ht["v16"][:, blk], v_blk[blk])  # SWDGE cast fp32->fp16
    nc.sync.dma_start(ht["a_sb"][:], a_h.rearrange("(t p) c -> p t c", p=128))
    nc.sync.dma_start(ht["w_sb"][:], w_h.rearrange("(t p) c -> p t c", p=128))


def build_head(tc, pools, consts, ht, out_h, blk_eng, prefetch_next, taps=None):
    nc = tc.nc
    temps, persist, ps_xt, ps_p1, ps_ctx, ps_o, ps_small = pools
    ident, stack2, dbias = consts[0], consts[1], consts[2]

    x_h = ht["x_h"]
    v16 = ht["v16"]
    phiT = ht["phiT"]
    nsq = ht["nsq"]
    rs = ht["rs"]
    qt0 = ht["qt0"]
    qt1 = ht["qt1"]
    kcp = ht["kcp"]
    g2 = ht["g2"]
    cc2 = ht["cc2"]
    cksb = ht["cksb"]
    a_sb = ht["a_sb"]
    w_sb = ht["w_sb"]

    out_blk = out_h.rearrange("(blk p a) c -> blk p (a c)", p=128, a=8)

    # ---------------- pass 1a: squared row norms -------------------------
    for blk in range(NBLK):
        sq16 = temps.tile([128, 8, 64], FP16, tag="sq16")
        nc.scalar.activation(sq16[:], x_h[:, blk], AF.Square)
        nc.vector.reduce_sum(nsq[:, blk], sq16[:], axis=AX.X)

    # ---------------- rsqrt via Quake + 2 Newton, two halves -------------
    ge = nc.vector
    magic = consts[3]
    for half in range(2):
        s = slice(half * 4, half * 4 + 4)
        nf = nsq[:, s].rearrange("p b a -> p (b a)")  # [128, 32] fp32
        r0f = qt0[:, s].rearrange("p b a -> p (b a)")
        t1f = qt1[:, s].rearrange("p b a -> p (b a)")
        rsf = rs[:, s].rearrange("p b a -> p (b a)")
        # r0 = bitcast(MAGIC - (bitcast(nsq) >> 1))
        ge.tensor_scalar(
            t1f.bitcast(U32), nf.bitcast(U32), 1, None, ALU.arith_shift_right
        )
        ge.tensor_tensor(
            r0f.bitcast(U32).rearrange("p (o a) -> p o a", o=1),
            bcast(magic[:], 32),
            t1f.bitcast(U32).rearrange("p (o a) -> p o a", o=1),
            ALU.subtract,
        )
        # Newton iter 1: r1 = r0 * (1.5 - 0.5 * nsq * r0^2)   -> rsf
        ge.tensor_tensor(t1f, r0f, r0f, ALU.mult)
        ge.tensor_tensor(t1f, t1f, nf, ALU.mult)
        ge.tensor_scalar(t1f, t1f, -0.5, 1.5, ALU.mult, op1=ALU.add)
        ge.tensor_tensor(rsf, r0f, t1f, ALU.mult)
        # Newton iter 2: rs = r1 * (1.5 - 0.5 * nsq * r1^2)
        ge.tensor_tensor(t1f, rsf, rsf, ALU.mult)
        ge.tensor_tensor(t1f, t1f, nf, ALU.mult)
        ge.tensor_scalar(t1f, t1f, -0.5, 1.5, ALU.mult, op1=ALU.add)
        ge.tensor_tensor(rsf, rsf, t1f, ALU.mult)

    # ---------------- G = anchor.T @ W_hash ; g2 block-diag fp16 ---------
    # (after pass 1a so the norm chain owns the early ACT/DVE slots)
    gk_ps = ps_small.tile([64, 65], FP32, tag="gk_ps")
    for t in range(2):
        nc.tensor.matmul(
            gk_ps[:, 0:64], a_sb[:, t, :], w_sb[:, t, :], start=(t == 0), stop=(t == 1)
        )
    nc.vector.memset(g2[:], 0.0)
    nc.vector.tensor_copy(g2[0:64, 0:64], gk_ps[:, 0:64])
    nc.vector.tensor_copy(g2[64:128, 64:128], gk_ps[:, 0:64])

    if taps is not None:
        nc.sync.dma_start(taps["rs"], rsf)
        nc.sync.dma_start(taps["nsq"], nf)

    # ---------------- pass 1b: phi production + ctx ----------------------
    # ctxAB: partitions 0:64 accumulate even-a ctx, 64:128 odd-a (col-tiled)
    ctxAB = ps_ctx.tile([128, 64], FP32, tag="ctxAB")
    for blk in range(NBLK):
        # xn = x * rs (per-token row scale), fp16, on the idle Pool engine
        xn = temps.tile([128, 8, 64], FP16, tag="xn")
        eng = nc.gpsimd
        eng.tensor_tensor(
            xn[:],
            x_h[:, blk],
            bcast(rs[:, blk].rearrange("p (a o) -> p a o", o=1), 64),
            ALU.mult,
        )

        # 4x PE transpose -> xt double-decker fp16
        xt_ps = ps_xt.tile([128, 512], FP16, tag="xt_ps")
        xn2 = xn[:].rearrange("p a c -> p (a c)")
        for j in range(4):
            nc.tensor.transpose(
                xt_ps[:, j * 128 : (j + 1) * 128],
                xn2[:, j * 128 : (j + 1) * 128],
                ident[:],
            )
        xt = temps.tile([128, 512], FP16, tag="xt")
        nc.vector.tensor_copy(xt[:], xt_ps[:])

        # s1T: phiT = tanh(G^T xhat^T), two concurrent row-tiled matmuls
        pT_ps = ps_p1.tile([128, 512], FP32, tag="pT_ps")
        nc.tensor.matmul(pT_ps[0:64, :], g2[0:64, 0:64], xt[0:64, :], start=True, stop=True)
        nc.tensor.matmul(pT_ps[64:128, :], g2[64:128, 64:128], xt[64:128, :], start=True, stop=True)
        # tanh with free-dim accumulation -> per-(deck,k) kcum contribution
        nc.scalar.activation(
            phiT[:, blk, :], pT_ps[:], AF.Tanh, accum_out=kcp[:, blk : blk + 1]
        )

        # s1N: phi natural [p, (a,k)] via 4 matmuls against block-diag g2
        pN_ps = ps_p1.tile([128, 512], FP32, tag="pN_ps")
        for j in range(4):
            nc.tensor.matmul(
                pN_ps[:, j * 128 : (j + 1) * 128],
                xt[:, j * 128 : (j + 1) * 128],
                g2[:],
                start=True,
                stop=True,
            )
        phi = temps.tile([128, 8, 64], FP16, tag="phi")
        nc.scalar.activation(phi[:].rearrange("p a c -> p (a c)"), pN_ps[:], AF.Tanh)
        if taps is not None and blk == 0:
            nc.sync.dma_start(taps["phi0"], phi[:].rearrange("p a c -> p (a c)"))
            nc.sync.dma_start(taps["phiT0"], phiT[:, 0, :])

        # ctx += phi_a.T @ v_a ; even a -> rows 0:64, odd a -> rows 64:128
        for a in range(8):
            half = a % 2
            nc.tensor.matmul(
                ctxAB[half * 64 : half * 64 + 64, :],
                phi[:, a, :],
                v16[:, blk, a, :],
                start=(blk == 0 and a == half),
                stop=(blk == NBLK - 1 and a == 6 + half),
                # sim's zero-region group check ignores partition base; the two
                # col-tiled halves are disjoint partition ranges on HW
                skip_group_check=True,
            )

    # prefetch the next head's inputs while this head's pass 2 runs
    if prefetch_next is not None:
        prefetch_next()

    # ---------------- head finalize: [ctx|kcum] + cc2 --------------------
    kc1 = temps.tile([128, 1], FP32, tag="kc1")
    nc.vector.reduce_sum(kc1[:], kcp[:], axis=AX.X)
    nc.scalar.copy(cksb[:, 0:64], ctxAB[:])
    nc.scalar.copy(cksb[:, 64:65], kc1[:])
    ck_ps = ps_small.tile([64, 65], FP32, tag="gk_ps")
    nc.tensor.matmul(ck_ps[:], stack2[:], cksb[:], start=True, stop=True)
    nc.vector.memset(cc2[:], 0.0)
    nc.scalar.copy(cc2[0:64, 0:65], ck_ps[:])
    nc.scalar.copy(cc2[64:128, 65:130], ck_ps[:])
    if taps is not None:
        nc.sync.dma_start(taps["cc2"], cc2[:])
        nc.sync.dma_start(taps["g2"], g2[:])

    # ---------------- pass 2 ---------------------------------------------
    # o_ab is one 2-bank psum tile; matmul outputs at col {0,130,512,642}
    # so numer/denom extraction is affine: off = 512*b + 65*g (g in 0..3).
    OFFS = (0, 130, 512, 642)
    for blk in range(NBLK):
        o_ab = ps_o.tile([128, 1024], FP32, tag="o_ab")
        for j in range(4):
            nc.tensor.matmul(
                o_ab[:, OFFS[j] : OFFS[j] + 130],
                phiT[:, blk, j * 128 : (j + 1) * 128],
                cc2[:],
                start=True,
                stop=True,
            )
        t = o_ab[:]
        denom = bass.AP(
            tensor=t.tensor, offset=t.offset + 64, ap=[t.ap[0], [512, 2], [65, 4]]
        )
        dnb = temps.tile([128, 8], FP32, tag="dnb")
        nc.scalar.activation(
            dnb[:].rearrange("p (b g) -> p b g", b=2), denom, AF.Identity, bias=dbias[:]
        )
        rec = temps.tile([128, 8], FP32, tag="rec")
        nc.vector.reciprocal(rec[:], dnb[:])
        # t_sb = 65*v + numer  (per psum bank: STT inputs are limited to 3D)
        t_sb = temps.tile([128, 8, 64], FP32, tag="t_sb")
        for b in range(2):
            numer_b = bass.AP(
                tensor=t.tensor, offset=t.offset + 512 * b, ap=[t.ap[0], [65, 4], [1, 64]]
            )
            nc.vector.scalar_tensor_tensor(
                out=t_sb[:, 4 * b : 4 * b + 4, :],
                in0=v16[:, blk, 4 * b : 4 * b + 4, :],
                scalar=float(BIAS),
                in1=numer_b,
                op0=ALU.mult,
                op1=ALU.add,
            )
        o_sb = temps.tile([128, 8, 64], FP32, tag="o_sb")
        eng = blk_eng(blk + 1)  # opposite parity from xn
        eng.tensor_tensor(
            o_sb[:],
            t_sb[:],
            bcast(rec[:].rearrange("p (a o) -> p a o", o=1), 64),
            ALU.mult,
        )
        nc.sync.dma_start(out_blk[blk], o_sb[:].rearrange("p a c -> p (a c)"))


def build_core(tc, pools, consts, qk_ap, v_ap, a_ap, w_ap, out_ap, heads):
    nc = tc.nc

    def blk_eng(blk):
        return nc.vector if blk % 2 == 0 else nc.gpsimd

    taps = None
    if DEBUG_TAPS:
        taps = {
            "rs": nc.dram_tensor("dbg_rs", (128, 64), FP32, kind="ExternalOutput").ap(),
            "nsq": nc.dram_tensor("dbg_nsq", (128, 64), FP32, kind="ExternalOutput").ap(),
            "phi0": nc.dram_tensor("dbg_phi0", (128, 512), FP16, kind="ExternalOutput").ap(),
            "phiT0": nc.dram_tensor("dbg_phiT0", (128, 512), FP16, kind="ExternalOutput").ap(),
            "cc2": nc.dram_tensor("dbg_cc2", (128, 130), FP16, kind="ExternalOutput").ap(),
            "g2": nc.dram_tensor("dbg_g2", (128, 128), FP16, kind="ExternalOutput").ap(),
        }

    persist = pools[1]
    tiles = [alloc_head_tiles(persist) for _ in range(heads)]
    emit_loads(nc, tiles[0], qk_ap[0], v_ap[0], a_ap[0], w_ap[0])
    for h in range(heads):
        if h + 1 < heads:
            hn = h + 1

            def prefetch_next(hn=hn):
                emit_loads(nc, tiles[hn], qk_ap[hn], v_ap[hn], a_ap[hn], w_ap[hn])
        else:
            prefetch_next = None
        build_head(
            tc,
            pools,
            consts,
            tiles[h],
            out_ap[h],
            blk_eng,
            prefetch_next,
            taps=taps if h == 0 else None,
        )


def build_bass(heads=HEADS_PER_CORE, repeat=1):
    nc = bacc.Bacc("TRN2", target_bir_lowering=False, debug=False, num_devices=8)
    hp = heads
    qk_ap = nc.dram_tensor("qk", (hp, N, C), FP32, kind="ExternalInput").ap()
    v_ap = nc.dram_tensor("v", (hp, N, C), FP32, kind="ExternalInput").ap()
    a_ap = nc.dram_tensor("anchor", (hp, 256, C), FP32, kind="ExternalInput").ap()
    w_ap = nc.dram_tensor("W_hash", (hp, 256, NBITS), FP32, kind="ExternalInput").ap()
    out_ap = nc.dram_tensor("out", (hp, N, C), FP32, kind="ExternalOutput").ap()

    with tile.TileContext(nc) as tc:
        with ExitStack() as ctx:
            singles = ctx.enter_context(tc.tile_pool(name="singles", bufs=1))
            temps = ctx.enter_context(tc.tile_pool(name="temps", bufs=3))
            persist = ctx.enter_context(tc.tile_pool(name="persist", bufs=2))
            ps_xt = ctx.enter_context(tc.tile_pool(name="ps_xt", bufs=2, space="PSUM"))
            ps_p1 = ctx.enter_context(tc.tile_pool(name="ps_p1", bufs=1, space="PSUM"))
            ps_ctx = ctx.enter_context(tc.tile_pool(name="ps_ctx", bufs=1, space="PSUM"))
            ps_o = ctx.enter_context(tc.tile_pool(name="ps_o", bufs=1, space="PSUM"))
            ps_small = ctx.enter_context(
                tc.tile_pool(name="ps_small", bufs=1, space="PSUM")
            )
            pools = (temps, persist, ps_xt, ps_p1, ps_ctx, ps_o, ps_small)

            ident = singles.tile([128, 128], FP16)
            make_identity(nc, ident[:])
            stack2 = singles.tile([128, 64], FP16)
            nc.scalar.copy(stack2[0:64, :], ident[0:64, 0:64])
            nc.scalar.copy(stack2[64:128, :], ident[0:64, 0:64])
            dbias = singles.tile([128, 1], FP32)
            nc.vector.memset(dbias[:], DENOM_BIAS)
            magic = singles.tile([128, 1], U32)
            nc.vector._memset_packed(magic[:], RSQRT_MAGIC)
            consts = (ident, stack2, dbias, magic)

            if repeat == 1:
                build_core(tc, pools, consts, qk_ap, v_ap, a_ap, w_ap, out_ap, heads)
            else:
                with tc.For_i(0, repeat, 1):
                    build_core(
                        tc, pools, consts, qk_ap, v_ap, a_ap, w_ap, out_ap, heads
                    )
    nc.compile()
    return nc


_NC_CACHE = None
_RUN_KWARGS = {}
_LAST_RESULTS = None


def kernel(qk, v, anchor, W_hash):
    global _NC_CACHE
    if _NC_CACHE is None:
        _NC_CACHE = build_bass()
    nc = _NC_CACHE

    qk = np.ascontiguousarray(qk, dtype=np.float32).reshape(B * H, N, C)
    v = np.ascontiguousarray(v, dtype=np.float32).reshape(B * H, N, C)
    anchor = np.ascontiguousarray(anchor, dtype=np.float32)
    W_hash = np.ascontiguousarray(W_hash, dtype=np.float32)

    in_maps = []
    for core in range(8):
        bh = np.arange(core * HEADS_PER_CORE, (core + 1) * HEADS_PER_CORE)
        h_idx = bh % H
        in_maps.append(
            {
                "qk": qk[bh],
                "v": v[bh],
                "anchor": np.ascontiguousarray(anchor[h_idx]),
                "W_hash": np.ascontiguousarray(W_hash[h_idx]),
            }
        )

    res = run_bass_kernel_spmd(nc, in_maps, core_ids=list(range(8)), **_RUN_KWARGS)
    global _LAST_RESULTS
    _LAST_RESULTS = res
    out = np.concatenate([res.results[c]["out"] for c in range(8)], axis=0)
    return out.reshape(B, H, N, C)


# revision 40
# speedup vs baseline: 1.4695x; 1.0439x over previous
"""Liteformer fast attention kernel for Trainium2 (8 NeuronCores), v2.

Math (per (b,h) head, N=8192 tokens, C=K=E=64, m=256 anchors):
    xhat = qk / ||qk||_row
    phi  = tanh((xhat @ anchor.T) @ W_hash) = tanh(xhat @ G),  G = anchor.T @ W_hash  [64,64]
    kcum = phi.sum(axis=0)                                  [64]
    ctx  = phi.T @ v                                        [64,64]
    out  = (phi @ ctx + 65*v) / (phi @ kcum + 8192*65)[:,None]

Sharding: B*H = 32 heads split 4-per-core across 8 cores (fully independent).

v2 engine plan (per 1024-token block; token(blk,p,a) = (blk*128+p)*8+a):
  SP  : x loads (8x256KB/head, prefetched), out stores (256KB/blk)
  POOL: v cast-loads fp32->fp16 (SWDGE), rsqrt via Quake+2 Newton, final mul
  ACT : Square (norms), Tanh phiT (+accum_out -> kcum), Tanh phi, denom bias
        -- single table set (exp_and_others), no table swaps
  DVE : reduce (norms), xn = x*rs (alternating with POOL), xt psum->sbuf copy,
        reciprocal, pass2 stt (65v+numer)
  PE  : 4 transposes, s1T row-tiled pair, s1N x4, ctx x8 col-tiled 2-way,
        pass2 x4 into one 2-bank psum tile (stride-65 numer/denom extraction)
"""

import os
import sys

sys.path.insert(0, "/opt/trn_rl_repo")

DEBUG_TAPS = os.environ.get("KERNEL_DEBUG_TAPS", "0") == "1"

from contextlib import ExitStack

import numpy as np

import concourse.bass as bass
import concourse.tile as tile
from concourse import bacc, mybir
from concourse.bass_utils import run_bass_kernel_spmd
from concourse.masks import make_identity

B, H, N, C = 2, 16, 8192, 64
NBITS = 64
BIAS = NBITS + 1  # 65
DENOM_BIAS = float(N) * BIAS  # 532480
HEADS_PER_CORE = (B * H) // 8  # 4
NBLK = N // 1024  # 8 blocks of 1024 tokens per head
FP32 = mybir.dt.float32
FP16 = mybir.dt.float16
U32 = mybir.dt.uint32
AX = mybir.AxisListType
AF = mybir.ActivationFunctionType
ALU = mybir.AluOpType

RSQRT_MAGIC = 0x5F3759DF


def bcast(ap, n):
    """Append a zero-stride broadcast dim of size n to an AP."""
    return bass.AP(tensor=ap.tensor, offset=ap.offset, ap=ap.ap + [[0, n]])


def alloc_head_tiles(persist):
    specs = {
        "x_h": ([128, NBLK, 8, 64], FP32),
        "v16": ([128, NBLK, 8, 64], FP16),
        "phiT": ([128, NBLK, 512], FP16),
        "nsq": ([128, NBLK, 8], FP32),
        "rs": ([128, NBLK, 8], FP32),
        "qt0": ([128, NBLK, 8], FP32),
        "qt1": ([128, NBLK, 8], FP32),
        "kcp": ([128, NBLK], FP32),
        "g2": ([128, 128], FP16),
        "cc2": ([128, 130], FP16),
        "cksb": ([128, 65], FP16),
        "a_sb": ([128, 2, 64], FP32),
        "w_sb": ([128, 2, 64], FP32),
    }
    return {
        k: persist.tile(shape, dt, tag=k, name=k) for k, (shape, dt) in specs.items()
    }


def emit_loads(nc, ht, qk_h, v_h, a_h, w_h):
    qk_blk = qk_h.rearrange("(blk p a) c -> blk p a c", p=128, a=8)
    v_blk = v_h.rearrange("(blk p a) c -> blk p a c", p=128, a=8)
    for blk in range(NBLK):
        nc.sync.dma_start(ht["x_h"][:, blk], qk_blk[blk])
        nc.gpsimd.dma_start(ht["v16"][:, blk], v_blk[blk])  # SWDGE cast fp32->fp16
    nc.sync.dma_start(ht["a_sb"][:], a_h.rearrange("(t p) c -> p t c", p=128))
    nc.sync.dma_start(ht["w_sb"][:], w_h.rearrange("(t p) c -> p t c", p=128))


def build_head(tc, pools, consts, ht, out_h, blk_eng, prefetch_next, taps=None):
    nc = tc.nc
    temps, persist, ps_xt, ps_p1, ps_ctx, ps_o, ps_small = pools
    ident, stack2, dbias = consts[0], consts[1], consts[2]

    x_h = ht["x_h"]
    v16 = ht["v16"]
    phiT = ht["phiT"]
    nsq = ht["nsq"]
    rs = ht["rs"]
    qt0 = ht["qt0"]
    qt1 = ht["qt1"]
    kcp = ht["kcp"]
    g2 = ht["g2"]
    cc2 = ht["cc2"]
    cksb = ht["cksb"]
    a_sb = ht["a_sb"]
    w_sb = ht["w_sb"]

    out_blk = out_h.rearrange("(blk p a) c -> blk p (a c)", p=128, a=8)

    # ---------------- pass 1a: squared row norms -------------------------
    for blk in range(NBLK):
        sq16 = temps.tile([128, 8, 64], FP16, tag="sq16")
        nc.scalar.activation(sq16[:], x_h[:, blk], AF.Square)
        nc.vector.reduce_sum(nsq[:, blk], sq16[:], axis=AX.X)

    # ---------------- rsqrt via Quake + 2 Newton, two halves -------------
    ge = nc.vector
    magic = consts[3]
    for half in range(2):
        s = slice(half * 4, half * 4 + 4)
        nf = nsq[:, s].rearrange("p b a -> p (b a)")  # [128, 32] fp32
        r0f = qt0[:, s].rearrange("p b a -> p (b a)")
        t1f = qt1[:, s].rearrange("p b a -> p (b a)")
        rsf = rs[:, s].rearrange("p b a -> p (b a)")
        # r0 = bitcast(MAGIC - (bitcast(nsq) >> 1))
        ge.tensor_scalar(
            t1f.bitcast(U32), nf.bitcast(U32), 1, None, ALU.arith_shift_right
        )
        ge.tensor_tensor(
            r0f.bitcast(U32).rearrange("p (o a) -> p o a", o=1),
            bcast(magic[:], 32),
            t1f.bitcast(U32).rearrange("p (o a) -> p o a", o=1),
            ALU.subtract,
        )
        # Newton iter 1: r1 = r0 * (1.5 - 0.5 * nsq * r0^2)   -> rsf
        ge.tensor_tensor(t1f, r0f, r0f, ALU.mult)
        ge.tensor_tensor(t1f, t1f, nf, ALU.mult)
        ge.tensor_scalar(t1f, t1f, -0.5, 1.5, ALU.mult, op1=ALU.add)
        ge.tensor_tensor(rsf, r0f, t1f, ALU.mult)
        # Newton iter 2: rs = r1 * (1.5 - 0.5 * nsq * r1^2)
        ge.tensor_tensor(t1f, rsf, rsf, ALU.mult)
        ge.tensor_tensor(t1f, t1f, nf, ALU.mult)
        ge.tensor_scalar(t1f, t1f, -0.5, 1.5, ALU.mult, op1=ALU.add)
        ge.tensor_tensor(rsf, rsf, t1f, ALU.mult)

    # ---------------- G = anchor.T @ W_hash ; g2 block-diag fp16 ---------
    # (after pass 1a so the norm chain owns the early ACT/DVE slots)
    gk_ps = ps_small.tile([64, 65], FP32, tag="gk_ps")
    for t in range(2):
        nc.tensor.matmul(
            gk_ps[:, 0:64], a_sb[:, t, :], w_sb[:, t, :], start=(t == 0), stop=(t == 1)
        )
    nc.vector.memset(g2[:], 0.0)
    nc.vector.tensor_copy(g2[0:64, 0:64], gk_ps[:, 0:64])
    nc.vector.tensor_copy(g2[64:128, 64:128], gk_ps[:, 0:64])

    if taps is not None:
        nc.sync.dma_start(taps["rs"], rsf)
        nc.sync.dma_start(taps["nsq"], nf)

    # ---------------- pass 1b: phi production + ctx ----------------------
    # ctxAB: partitions 0:64 accumulate even-a ctx, 64:128 odd-a (col-tiled)
    ctxAB = ps_ctx.tile([128, 64], FP32, tag="ctxAB")
    for blk in range(NBLK):
        # xn = x * rs (per-token row scale), fp16, on the idle Pool engine
        xn = temps.tile([128, 8, 64], FP16, tag="xn")
        eng = nc.gpsimd
        eng.tensor_tensor(
            xn[:],
            x_h[:, blk],
            bcast(rs[:, blk].rearrange("p (a o) -> p a o", o=1), 64),
            ALU.mult,
        )

        # 4x PE transpose -> xt double-decker fp16
        xt_ps = ps_xt.tile([128, 512], FP16, tag="xt_ps")
        xn2 = xn[:].rearrange("p a c -> p (a c)")
        for j in range(4):
            nc.tensor.transpose(
                xt_ps[:, j * 128 : (j + 1) * 128],
                xn2[:, j * 128 : (j + 1) * 128],
                ident[:],
            )
        xt = temps.tile([128, 512], FP16, tag="xt")
        if blk % 2 == 0:
            nc.vector.tensor_copy(xt[:], xt_ps[:])
        else:
            nc.scalar.copy(xt[:], xt_ps[:])

        # s1T: phiT = tanh(G^T xhat^T), two concurrent row-tiled matmuls
        pT_ps = ps_p1.tile([128, 512], FP32, tag="pT_ps")
        nc.tensor.matmul(pT_ps[0:64, :], g2[0:64, 0:64], xt[0:64, :], start=True, stop=True)
        nc.tensor.matmul(pT_ps[64:128, :], g2[64:128, 64:128], xt[64:128, :], start=True, stop=True)
        # tanh with free-dim accumulation -> per-(deck,k) kcum contribution
        nc.scalar.activation(
            phiT[:, blk, :], pT_ps[:], AF.Tanh, accum_out=kcp[:, blk : blk + 1]
        )

        # s1N: phi natural [p, (a,k)] via 4 matmuls against block-diag g2
        pN_ps = ps_p1.tile([128, 512], FP32, tag="pN_ps")
        for j in range(4):
            nc.tensor.matmul(
                pN_ps[:, j * 128 : (j + 1) * 128],
                xt[:, j * 128 : (j + 1) * 128],
                g2[:],
                start=True,
                stop=True,
            )
        phi = temps.tile([128, 8, 64], FP16, tag="phi")
        nc.scalar.activation(phi[:].rearrange("p a c -> p (a c)"), pN_ps[:], AF.Tanh)
        if taps is not None and blk == 0:
            nc.sync.dma_start(taps["phi0"], phi[:].rearrange("p a c -> p (a c)"))
            nc.sync.dma_start(taps["phiT0"], phiT[:, 0, :])

        # ctx += phi_a.T @ v_a ; even a -> rows 0:64, odd a -> rows 64:128
        for a in range(8):
            half = a % 2
            nc.tensor.matmul(
                ctxAB[half * 64 : half * 64 + 64, :],
                phi[:, a, :],
                v16[:, blk, a, :],
                start=(blk == 0 and a == half),
                stop=(blk == NBLK - 1 and a == 6 + half),
                # sim's zero-region group check ignores partition base; the two
                # col-tiled halves are disjoint partition ranges on HW
                skip_group_check=True,
            )

    # prefetch the next head's inputs while this head's pass 2 runs
    if prefetch_next is not None:
        prefetch_next()

    # ---------------- head finalize: [ctx|kcum] + cc2 --------------------
    kc1 = temps.tile([128, 1], FP32, tag="kc1")
    nc.vector.reduce_sum(kc1[:], kcp[:], axis=AX.X)
    nc.scalar.copy(cksb[:, 0:64], ctxAB[:])
    nc.scalar.copy(cksb[:, 64:65], kc1[:])
    ck_ps = ps_small.tile([64, 65], FP32, tag="gk_ps")
    nc.tensor.matmul(ck_ps[:], stack2[:], cksb[:], start=True, stop=True)
    nc.vector.memset(cc2[:], 0.0)
    nc.scalar.copy(cc2[0:64, 0:65], ck_ps[:])
    nc.scalar.copy(cc2[64:128, 65:130], ck_ps[:])
    if taps is not None:
        nc.sync.dma_start(taps["cc2"], cc2[:])
        nc.sync.dma_start(taps["g2"], g2[:])

    # ---------------- pass 2 ---------------------------------------------
    # o_ab is one 2-bank psum tile; matmul outputs at col {0,130,512,642}
    # so numer/denom extraction is affine: off = 512*b + 65*g (g in 0..3).
    OFFS = (0, 130, 512, 642)
    for blk in range(NBLK):
        o_ab = ps_o.tile([128, 1024], FP32, tag="o_ab")
        for j in range(4):
            nc.tensor.matmul(
                o_ab[:, OFFS[j] : OFFS[j] + 130],
                phiT[:, blk, j * 128 : (j + 1) * 128],
                cc2[:],
                start=True,
                stop=True,
            )
        t = o_ab[:]
        denom = bass.AP(
            tensor=t.tensor, offset=t.offset + 64, ap=[t.ap[0], [512, 2], [65, 4]]
        )
        dnb = temps.tile([128, 8], FP32, tag="dnb")
        nc.scalar.activation(
            dnb[:].rearrange("p (b g) -> p b g", b=2), denom, AF.Identity, bias=dbias[:]
        )
        rec = temps.tile([128, 8], FP32, tag="rec")
        nc.vector.reciprocal(rec[:], dnb[:])
        # t_sb = 65*v + numer  (per psum bank: STT inputs are limited to 3D)
        t_sb = temps.tile([128, 8, 64], FP32, tag="t_sb")
        for b in range(2):
            numer_b = bass.AP(
                tensor=t.tensor, offset=t.offset + 512 * b, ap=[t.ap[0], [65, 4], [1, 64]]
            )
            nc.vector.scalar_tensor_tensor(
                out=t_sb[:, 4 * b : 4 * b + 4, :],
                in0=v16[:, blk, 4 * b : 4 * b + 4, :],
                scalar=float(BIAS),
                in1=numer_b,
                op0=ALU.mult,
                op1=ALU.add,
            )
        o_sb = temps.tile([128, 8, 64], FP32, tag="o_sb")
        eng = blk_eng(blk + 1)  # opposite parity from xn
        eng.tensor_tensor(
            o_sb[:],
            t_sb[:],
            bcast(rec[:].rearrange("p (a o) -> p a o", o=1), 64),
            ALU.mult,
        )
        nc.sync.dma_start(out_blk[blk], o_sb[:].rearrange("p a c -> p (a c)"))


def build_core(tc, pools, consts, qk_ap, v_ap, a_ap, w_ap, out_ap, heads):
    nc = tc.nc

    def blk_eng(blk):
        return nc.vector if blk % 2 == 0 else nc.gpsimd

    taps = None
    if DEBUG_TAPS:
        taps = {
            "rs": nc.dram_tensor("dbg_rs", (128, 64), FP32, kind="ExternalOutput").ap(),
            "nsq": nc.dram_tensor("dbg_nsq", (128, 64), FP32, kind="ExternalOutput").ap(),
            "phi0": nc.dram_tensor("dbg_phi0", (128, 512), FP16, kind="ExternalOutput").ap(),
            "phiT0": nc.dram_tensor("dbg_phiT0", (128, 512), FP16, kind="ExternalOutput").ap(),
            "cc2": nc.dram_tensor("dbg_cc2", (128, 130), FP16, kind="ExternalOutput").ap(),
            "g2": nc.dram_tensor("dbg_g2", (128, 128), FP16, kind="ExternalOutput").ap(),
        }

    persist = pools[1]
    tiles = [alloc_head_tiles(persist) for _ in range(heads)]
    emit_loads(nc, tiles[0], qk_ap[0], v_ap[0], a_ap[0], w_ap[0])
    for h in range(heads):
        if h + 1 < heads:
            hn = h + 1

            def prefetch_next(hn=hn):
                emit_loads(nc, tiles[hn], qk_ap[hn], v_ap[hn], a_ap[hn], w_ap[hn])
        else:
            prefetch_next = None
        build_head(
            tc,
            pools,
            consts,
            tiles[h],
            out_ap[h],
            blk_eng,
            prefetch_next,
            taps=taps if h == 0 else None,
        )


def build_bass(heads=HEADS_PER_CORE, repeat=1):
    nc = bacc.Bacc("TRN2", target_bir_lowering=False, debug=False, num_devices=8)
    hp = heads
    qk_ap = nc.dram_tensor("qk", (hp, N, C), FP32, kind="ExternalInput").ap()
    v_ap = nc.dram_tensor("v", (hp, N, C), FP32, kind="ExternalInput").ap()
    a_ap = nc.dram_tensor("anchor", (hp, 256, C), FP32, kind="ExternalInput").ap()
    w_ap = nc.dram_tensor("W_hash", (hp, 256, NBITS), FP32, kind="ExternalInput").ap()
    out_ap = nc.dram_tensor("out", (hp, N, C), FP32, kind="ExternalOutput").ap()

    with tile.TileContext(nc) as tc:
        with ExitStack() as ctx:
            singles = ctx.enter_context(tc.tile_pool(name="singles", bufs=1))
            temps = ctx.enter_context(tc.tile_pool(name="temps", bufs=4))
            persist = ctx.enter_context(tc.tile_pool(name="persist", bufs=2))
            ps_xt = ctx.enter_context(tc.tile_pool(name="ps_xt", bufs=2, space="PSUM"))
            ps_p1 = ctx.enter_context(tc.tile_pool(name="ps_p1", bufs=1, space="PSUM"))
            ps_ctx = ctx.enter_context(tc.tile_pool(name="ps_ctx", bufs=1, space="PSUM"))
            ps_o = ctx.enter_context(tc.tile_pool(name="ps_o", bufs=1, space="PSUM"))
            ps_small = ctx.enter_context(
                tc.tile_pool(name="ps_small", bufs=1, space="PSUM")
            )
            pools = (temps, persist, ps_xt, ps_p1, ps_ctx, ps_o, ps_small)

            ident = singles.tile([128, 128], FP16)
            make_identity(nc, ident[:])
            stack2 = singles.tile([128, 64], FP16)
            nc.scalar.copy(stack2[0:64, :], ident[0:64, 0:64])
            nc.scalar.copy(stack2[64:128, :], ident[0:64, 0:64])
            dbias = singles.tile([128, 1], FP32)
            nc.vector.memset(dbias[:], DENOM_BIAS)
            magic = singles.tile([128, 1], U32)
            nc.vector._memset_packed(magic[:], RSQRT_MAGIC)
            ones1 = singles.tile([128, 1], FP32)
            nc.vector.memset(ones1[:], 1.0)
            consts = (ident, stack2, dbias, magic, ones1)

            if repeat == 1:
                build_core(tc, pools, consts, qk_ap, v_ap, a_ap, w_ap, out_ap, heads)
            else:
                with tc.For_i(0, repeat, 1):
                    build_core(
                        tc, pools, consts, qk_ap, v_ap, a_ap, w_ap, out_ap, heads
                    )
    nc.compile()
    return nc


_NC_CACHE = None
_RUN_KWARGS = {}
_LAST_RESULTS = None


def kernel(qk, v, anchor, W_hash):
    global _NC_CACHE
    if _NC_CACHE is None:
        _NC_CACHE = build_bass()
    nc = _NC_CACHE

    qk = np.ascontiguousarray(qk, dtype=np.float32).reshape(B * H, N, C)
    v = np.ascontiguousarray(v, dtype=np.float32).reshape(B * H, N, C)
    anchor = np.ascontiguousarray(anchor, dtype=np.float32)
    W_hash = np.ascontiguousarray(W_hash, dtype=np.float32)

    in_maps = []
    for core in range(8):
        bh = np.arange(core * HEADS_PER_CORE, (core + 1) * HEADS_PER_CORE)
        h_idx = bh % H
        in_maps.append(
            {
                "qk": qk[bh],
                "v": v[bh],
                "anchor": np.ascontiguousarray(anchor[h_idx]),
                "W_hash": np.ascontiguousarray(W_hash[h_idx]),
            }
        )

    res = run_bass_kernel_spmd(nc, in_maps, core_ids=list(range(8)), **_RUN_KWARGS)
    global _LAST_RESULTS
    _LAST_RESULTS = res
    out = np.concatenate([res.results[c]["out"] for c in range(8)], axis=0)
    return out.reshape(B, H, N, C)


# revision 42
# speedup vs baseline: 1.5422x; 1.0495x over previous
"""Liteformer fast attention kernel for Trainium2 (8 NeuronCores), v2.

Math (per (b,h) head, N=8192 tokens, C=K=E=64, m=256 anchors):
    xhat = qk / ||qk||_row
    phi  = tanh((xhat @ anchor.T) @ W_hash) = tanh(xhat @ G),  G = anchor.T @ W_hash  [64,64]
    kcum = phi.sum(axis=0)                                  [64]
    ctx  = phi.T @ v                                        [64,64]
    out  = (phi @ ctx + 65*v) / (phi @ kcum + 8192*65)[:,None]

Sharding: B*H = 32 heads split 4-per-core across 8 cores (fully independent).

v2 engine plan (per 1024-token block; token(blk,p,a) = (blk*128+p)*8+a):
  SP  : x loads (8x256KB/head, prefetched), out stores (256KB/blk)
  POOL: v cast-loads fp32->fp16 (SWDGE), rsqrt via Quake+2 Newton, final mul
  ACT : Square (norms), Tanh phiT (+accum_out -> kcum), Tanh phi, denom bias
        -- single table set (exp_and_others), no table swaps
  DVE : reduce (norms), xn = x*rs (alternating with POOL), xt psum->sbuf copy,
        reciprocal, pass2 stt (65v+numer)
  PE  : 4 transposes, s1T row-tiled pair, s1N x4, ctx x8 col-tiled 2-way,
        pass2 x4 into one 2-bank psum tile (stride-65 numer/denom extraction)
"""

import os
import sys

sys.path.insert(0, "/opt/trn_rl_repo")

DEBUG_TAPS = os.environ.get("KERNEL_DEBUG_TAPS", "0") == "1"

from contextlib import ExitStack

import numpy as np

import concourse.bass as bass
import concourse.tile as tile
from concourse import bacc, mybir
from concourse.bass_utils import run_bass_kernel_spmd
from concourse.masks import make_identity

B, H, N, C = 2, 16, 8192, 64
NBITS = 64
BIAS = NBITS + 1  # 65
DENOM_BIAS = float(N) * BIAS  # 532480
HEADS_PER_CORE = (B * H) // 8  # 4
NBLK = N // 1024  # 8 blocks of 1024 tokens per head
FP32 = mybir.dt.float32
FP16 = mybir.dt.float16
U32 = mybir.dt.uint32
AX = mybir.AxisListType
AF = mybir.ActivationFunctionType
ALU = mybir.AluOpType

RSQRT_MAGIC = 0x5F3759DF


def bcast(ap, n):
    """Append a zero-stride broadcast dim of size n to an AP."""
    return bass.AP(tensor=ap.tensor, offset=ap.offset, ap=ap.ap + [[0, n]])


def alloc_head_tiles(persist):
    specs = {
        "x_h": ([128, NBLK, 8, 64], FP32),
        "v16": ([128, NBLK, 8, 64], FP16),
        "phiT": ([128, NBLK, 512], FP16),
        "nsq": ([128, NBLK, 8], FP32),
        "rs": ([128, NBLK, 8], FP32),
        "qt0": ([128, NBLK, 8], FP32),
        "qt1": ([128, NBLK, 8], FP32),
        "kcp": ([128, NBLK], FP32),
        "g2": ([128, 128], FP16),
        "cc2": ([128, 130], FP16),
        "cksb": ([128, 65], FP16),
        "a_sb": ([128, 2, 64], FP32),
        "w_sb": ([128, 2, 64], FP32),
    }
    return {
        k: persist.tile(shape, dt, tag=k, name=k) for k, (shape, dt) in specs.items()
    }


def emit_loads(nc, ht, qk_h, v_h, a_h, w_h):
    qk_blk = qk_h.rearrange("(blk p a) c -> blk p a c", p=128, a=8)
    v_blk = v_h.rearrange("(blk p a) c -> blk p a c", p=128, a=8)
    for blk in range(NBLK):
        nc.sync.dma_start(ht["x_h"][:, blk], qk_blk[blk])
        nc.gpsimd.dma_start(ht["v16"][:, blk], v_blk[blk])  # SWDGE cast fp32->fp16
    nc.sync.dma_start(ht["a_sb"][:], a_h.rearrange("(t p) c -> p t c", p=128))
    nc.sync.dma_start(ht["w_sb"][:], w_h.rearrange("(t p) c -> p t c", p=128))


def build_head(tc, pools, consts, ht, out_h, blk_eng, prefetch_next, taps=None):
    nc = tc.nc
    temps, persist, ps_xt, ps_p1, ps_ctx, ps_o, ps_small = pools
    ident, stack2, dbias = consts[0], consts[1], consts[2]

    x_h = ht["x_h"]
    v16 = ht["v16"]
    phiT = ht["phiT"]
    nsq = ht["nsq"]
    rs = ht["rs"]
    qt0 = ht["qt0"]
    qt1 = ht["qt1"]
    kcp = ht["kcp"]
    g2 = ht["g2"]
    cc2 = ht["cc2"]
    cksb = ht["cksb"]
    a_sb = ht["a_sb"]
    w_sb = ht["w_sb"]

    out_blk = out_h.rearrange("(blk p a) c -> blk p (a c)", p=128, a=8)

    # ---------------- pass 1a: squared row norms -------------------------
    for blk in range(NBLK):
        sq16 = temps.tile([128, 8, 64], FP16, tag="sq16")
        nc.scalar.activation(sq16[:], x_h[:, blk], AF.Square)
        nc.vector.reduce_sum(nsq[:, blk], sq16[:], axis=AX.X)

    # ---------------- rsqrt via Quake + 2 Newton, two halves -------------
    ge = nc.vector
    magic = consts[3]
    for half in range(2):
        s = slice(half * 4, half * 4 + 4)
        nf = nsq[:, s].rearrange("p b a -> p (b a)")  # [128, 32] fp32
        r0f = qt0[:, s].rearrange("p b a -> p (b a)")
        t1f = qt1[:, s].rearrange("p b a -> p (b a)")
        rsf = rs[:, s].rearrange("p b a -> p (b a)")
        # r0 = bitcast(MAGIC - (bitcast(nsq) >> 1))
        ge.tensor_scalar(
            t1f.bitcast(U32), nf.bitcast(U32), 1, None, ALU.arith_shift_right
        )
        ge.tensor_tensor(
            r0f.bitcast(U32).rearrange("p (o a) -> p o a", o=1),
            bcast(magic[:], 32),
            t1f.bitcast(U32).rearrange("p (o a) -> p o a", o=1),
            ALU.subtract,
        )
        # Newton iter 1: r1 = r0 * (1.5 - 0.5 * nsq * r0^2)   -> rsf
        ge.tensor_tensor(t1f, r0f, r0f, ALU.mult)
        ge.tensor_tensor(t1f, t1f, nf, ALU.mult)
        ge.tensor_scalar(t1f, t1f, -0.5, 1.5, ALU.mult, op1=ALU.add)
        ge.tensor_tensor(rsf, r0f, t1f, ALU.mult)
        # Newton iter 2 (Halley-lite): one more refinement keeps rs to ~2e-3
        # rel err after iter 1; iter 2 would give 5e-6 but costs 4 DVE ops on
        # the busiest engine. Error budget (2e-2) has >10x headroom at 1 iter.

    # ---------------- G = anchor.T @ W_hash ; g2 block-diag fp16 ---------
    # (after pass 1a so the norm chain owns the early ACT/DVE slots)
    gk_ps = ps_small.tile([64, 65], FP32, tag="gk_ps")
    for t in range(2):
        nc.tensor.matmul(
            gk_ps[:, 0:64], a_sb[:, t, :], w_sb[:, t, :], start=(t == 0), stop=(t == 1)
        )
    nc.vector.memset(g2[:], 0.0)
    nc.vector.tensor_copy(g2[0:64, 0:64], gk_ps[:, 0:64])
    nc.vector.tensor_copy(g2[64:128, 64:128], gk_ps[:, 0:64])

    if taps is not None:
        nc.sync.dma_start(taps["rs"], rsf)
        nc.sync.dma_start(taps["nsq"], nf)

    # ---------------- pass 1b: phi production + ctx ----------------------
    # ctxAB: partitions 0:64 accumulate even-a ctx, 64:128 odd-a (col-tiled)
    ctxAB = ps_ctx.tile([128, 64], FP32, tag="ctxAB")
    for blk in range(NBLK):
        # xn = x * rs (per-token row scale), fp16, on the idle Pool engine
        xn = temps.tile([128, 8, 64], FP16, tag="xn")
        eng = nc.gpsimd
        eng.tensor_tensor(
            xn[:],
            x_h[:, blk],
            bcast(rs[:, blk].rearrange("p (a o) -> p a o", o=1), 64),
            ALU.mult,
        )

        # 4x PE transpose -> xt double-decker fp16
        xt_ps = ps_xt.tile([128, 512], FP16, tag="xt_ps")
        xn2 = xn[:].rearrange("p a c -> p (a c)")
        for j in range(4):
            nc.tensor.transpose(
                xt_ps[:, j * 128 : (j + 1) * 128],
                xn2[:, j * 128 : (j + 1) * 128],
                ident[:],
            )
        xt = temps.tile([128, 512], FP16, tag="xt")
        if blk % 2 == 0:
            nc.vector.tensor_copy(xt[:], xt_ps[:])
        else:
            nc.scalar.copy(xt[:], xt_ps[:])

        # s1T: phiT = tanh(G^T xhat^T), two concurrent row-tiled matmuls
        pT_ps = ps_p1.tile([128, 512], FP32, tag="pT_ps")
        nc.tensor.matmul(pT_ps[0:64, :], g2[0:64, 0:64], xt[0:64, :], start=True, stop=True)
        nc.tensor.matmul(pT_ps[64:128, :], g2[64:128, 64:128], xt[64:128, :], start=True, stop=True)
        # tanh with free-dim accumulation -> per-(deck,k) kcum contribution
        nc.scalar.activation(
            phiT[:, blk, :], pT_ps[:], AF.Tanh, accum_out=kcp[:, blk : blk + 1]
        )

        # s1N: phi natural [p, (a,k)] via 4 matmuls against block-diag g2
        pN_ps = ps_p1.tile([128, 512], FP32, tag="pN_ps")
        for j in range(4):
            nc.tensor.matmul(
                pN_ps[:, j * 128 : (j + 1) * 128],
                xt[:, j * 128 : (j + 1) * 128],
                g2[:],
                start=True,
                stop=True,
            )
        phi = temps.tile([128, 8, 64], FP16, tag="phi")
        nc.scalar.activation(phi[:].rearrange("p a c -> p (a c)"), pN_ps[:], AF.Tanh)
        if taps is not None and blk == 0:
            nc.sync.dma_start(taps["phi0"], phi[:].rearrange("p a c -> p (a c)"))
            nc.sync.dma_start(taps["phiT0"], phiT[:, 0, :])

        # ctx += phi_a.T @ v_a ; even a -> rows 0:64, odd a -> rows 64:128
        for a in range(8):
            half = a % 2
            nc.tensor.matmul(
                ctxAB[half * 64 : half * 64 + 64, :],
                phi[:, a, :],
                v16[:, blk, a, :],
                start=(blk == 0 and a == half),
                stop=(blk == NBLK - 1 and a == 6 + half),
                # sim's zero-region group check ignores partition base; the two
                # col-tiled halves are disjoint partition ranges on HW
                skip_group_check=True,
            )

    # prefetch the next head's inputs while this head's pass 2 runs
    if prefetch_next is not None:
        prefetch_next()

    # ---------------- head finalize: [ctx|kcum] + cc2 --------------------
    kc1 = temps.tile([128, 1], FP32, tag="kc1")
    nc.vector.reduce_sum(kc1[:], kcp[:], axis=AX.X)
    nc.scalar.copy(cksb[:, 0:64], ctxAB[:])
    nc.scalar.copy(cksb[:, 64:65], kc1[:])
    ck_ps = ps_small.tile([64, 65], FP32, tag="gk_ps")
    nc.tensor.matmul(ck_ps[:], stack2[:], cksb[:], start=True, stop=True)
    nc.vector.memset(cc2[:], 0.0)
    nc.scalar.copy(cc2[0:64, 0:65], ck_ps[:])
    nc.scalar.copy(cc2[64:128, 65:130], ck_ps[:])
    if taps is not None:
        nc.sync.dma_start(taps["cc2"], cc2[:])
        nc.sync.dma_start(taps["g2"], g2[:])

    # ---------------- pass 2 ---------------------------------------------
    # o_ab is one 2-bank psum tile; matmul outputs at col {0,130,512,642}
    # so numer/denom extraction is affine: off = 512*b + 65*g (g in 0..3).
    OFFS = (0, 130, 512, 642)
    for blk in range(NBLK):
        o_ab = ps_o.tile([128, 1024], FP32, tag="o_ab")
        for j in range(4):
            nc.tensor.matmul(
                o_ab[:, OFFS[j] : OFFS[j] + 130],
                phiT[:, blk, j * 128 : (j + 1) * 128],
                cc2[:],
                start=True,
                stop=True,
            )
        t = o_ab[:]
        denom = bass.AP(
            tensor=t.tensor, offset=t.offset + 64, ap=[t.ap[0], [512, 2], [65, 4]]
        )
        dnb = temps.tile([128, 8], FP32, tag="dnb")
        nc.scalar.activation(
            dnb[:].rearrange("p (b g) -> p b g", b=2), denom, AF.Identity, bias=dbias[:]
        )
        rec = temps.tile([128, 8], FP32, tag="rec")
        nc.vector.reciprocal(rec[:], dnb[:])
        # t_sb = 65*v + numer  (per psum bank: STT inputs are limited to 3D)
        t_sb = temps.tile([128, 8, 64], FP32, tag="t_sb")
        for b in range(2):
            numer_b = bass.AP(
                tensor=t.tensor, offset=t.offset + 512 * b, ap=[t.ap[0], [65, 4], [1, 64]]
            )
            nc.vector.scalar_tensor_tensor(
                out=t_sb[:, 4 * b : 4 * b + 4, :],
                in0=v16[:, blk, 4 * b : 4 * b + 4, :],
                scalar=float(BIAS),
                in1=numer_b,
                op0=ALU.mult,
                op1=ALU.add,
            )
        o_sb = temps.tile([128, 8, 64], FP32, tag="o_sb")
        eng = blk_eng(blk + 1)  # opposite parity from xn
        eng.tensor_tensor(
            o_sb[:],
            t_sb[:],
            bcast(rec[:].rearrange("p (a o) -> p a o", o=1), 64),
            ALU.mult,
        )
        nc.sync.dma_start(out_blk[blk], o_sb[:].rearrange("p a c -> p (a c)"))


def build_core(tc, pools, consts, qk_ap, v_ap, a_ap, w_ap, out_ap, heads):
    nc = tc.nc

    def blk_eng(blk):
        return nc.vector if blk % 2 == 0 else nc.gpsimd

    taps = None
    if DEBUG_TAPS:
        taps = {
            "rs": nc.dram_tensor("dbg_rs", (128, 64), FP32, kind="ExternalOutput").ap(),
            "nsq": nc.dram_tensor("dbg_nsq", (128, 64), FP32, kind="ExternalOutput").ap(),
            "phi0": nc.dram_tensor("dbg_phi0", (128, 512), FP16, kind="ExternalOutput").ap(),
            "phiT0": nc.dram_tensor("dbg_phiT0", (128, 512), FP16, kind="ExternalOutput").ap(),
            "cc2": nc.dram_tensor("dbg_cc2", (128, 130), FP16, kind="ExternalOutput").ap(),
            "g2": nc.dram_tensor("dbg_g2", (128, 128), FP16, kind="ExternalOutput").ap(),
        }

    persist = pools[1]
    tiles = [alloc_head_tiles(persist) for _ in range(heads)]
    emit_loads(nc, tiles[0], qk_ap[0], v_ap[0], a_ap[0], w_ap[0])
    for h in range(heads):
        if h + 1 < heads:
            hn = h + 1

            def prefetch_next(hn=hn):
                emit_loads(nc, tiles[hn], qk_ap[hn], v_ap[hn], a_ap[hn], w_ap[hn])
        else:
            prefetch_next = None
        build_head(
            tc,
            pools,
            consts,
            tiles[h],
            out_ap[h],
            blk_eng,
            prefetch_next,
            taps=taps if h == 0 else None,
        )


def build_bass(heads=HEADS_PER_CORE, repeat=1):
    nc = bacc.Bacc("TRN2", target_bir_lowering=False, debug=False, num_devices=8)
    hp = heads
    qk_ap = nc.dram_tensor("qk", (hp, N, C), FP32, kind="ExternalInput").ap()
    v_ap = nc.dram_tensor("v", (hp, N, C), FP32, kind="ExternalInput").ap()
    a_ap = nc.dram_tensor("anchor", (hp, 256, C), FP32, kind="ExternalInput").ap()
    w_ap = nc.dram_tensor("W_hash", (hp, 256, NBITS), FP32, kind="ExternalInput").ap()
    out_ap = nc.dram_tensor("out", (hp, N, C), FP32, kind="ExternalOutput").ap()

    with tile.TileContext(nc) as tc:
        with ExitStack() as ctx:
            singles = ctx.enter_context(tc.tile_pool(name="singles", bufs=1))
            temps = ctx.enter_context(tc.tile_pool(name="temps", bufs=4))
            persist = ctx.enter_context(tc.tile_pool(name="persist", bufs=3))
            ps_xt = ctx.enter_context(tc.tile_pool(name="ps_xt", bufs=2, space="PSUM"))
            ps_p1 = ctx.enter_context(tc.tile_pool(name="ps_p1", bufs=1, space="PSUM"))
            ps_ctx = ctx.enter_context(tc.tile_pool(name="ps_ctx", bufs=1, space="PSUM"))
            ps_o = ctx.enter_context(tc.tile_pool(name="ps_o", bufs=1, space="PSUM"))
            ps_small = ctx.enter_context(
                tc.tile_pool(name="ps_small", bufs=1, space="PSUM")
            )
            pools = (temps, persist, ps_xt, ps_p1, ps_ctx, ps_o, ps_small)

            ident = singles.tile([128, 128], FP16)
            make_identity(nc, ident[:])
            stack2 = singles.tile([128, 64], FP16)
            nc.scalar.copy(stack2[0:64, :], ident[0:64, 0:64])
            nc.scalar.copy(stack2[64:128, :], ident[0:64, 0:64])
            dbias = singles.tile([128, 1], FP32)
            nc.vector.memset(dbias[:], DENOM_BIAS)
            magic = singles.tile([128, 1], U32)
            nc.vector._memset_packed(magic[:], RSQRT_MAGIC)
            ones1 = singles.tile([128, 1], FP32)
            nc.vector.memset(ones1[:], 1.0)
            consts = (ident, stack2, dbias, magic, ones1)

            if repeat == 1:
                build_core(tc, pools, consts, qk_ap, v_ap, a_ap, w_ap, out_ap, heads)
            else:
                with tc.For_i(0, repeat, 1):
                    build_core(
                        tc, pools, consts, qk_ap, v_ap, a_ap, w_ap, out_ap, heads
                    )
    nc.compile()
    return nc


_NC_CACHE = None
_RUN_KWARGS = {}
_LAST_RESULTS = None


def kernel(qk, v, anchor, W_hash):
    global _NC_CACHE
    if _NC_CACHE is None:
        _NC_CACHE = build_bass()
    nc = _NC_CACHE

    qk = np.ascontiguousarray(qk, dtype=np.float32).reshape(B * H, N, C)
    v = np.ascontiguousarray(v, dtype=np.float32).reshape(B * H, N, C)
    anchor = np.ascontiguousarray(anchor, dtype=np.float32)
    W_hash = np.ascontiguousarray(W_hash, dtype=np.float32)

    in_maps = []
    for core in range(8):
        bh = np.arange(core * HEADS_PER_CORE, (core + 1) * HEADS_PER_CORE)
        h_idx = bh % H
        in_maps.append(
            {
                "qk": qk[bh],
                "v": v[bh],
                "anchor": np.ascontiguousarray(anchor[h_idx]),
                "W_hash": np.ascontiguousarray(W_hash[h_idx]),
            }
        )

    res = run_bass_kernel_spmd(nc, in_maps, core_ids=list(range(8)), **_RUN_KWARGS)
    global _LAST_RESULTS
    _LAST_RESULTS = res
    out = np.concatenate([res.results[c]["out"] for c in range(8)], axis=0)
    return out.reshape(B, H, N, C)
